# revision 4
# baseline (speedup 1.0000x reference)
"""Multi-head causal+padded attention on 8 TRN2 NeuronCores.

Strategy: data-parallel over batch (8 batches -> 8 cores, no collectives).
Per core, everything is computed in a transposed layout so that no PE
transposes of the attention matrix are needed:

  QT[h] = (q Wq^T)^T slice  [e=128, tq]     KT[h] likewise
  V[kc] = (k Wv^T) row-chunk [tk=128, he]   (natural layout)
  S^T(kc,:) = KT[h][:,kc]^T-block matmuls   [tk-part, tq-free]
  A^T = exp(s * S^T + mk_bias)              (pad mask folded into exp bias,
                                             causal diag masked by a bf16
                                             identity x (-1e30 tri) matmul
                                             injected into PSUM)
  row sums  = ones^T @ A^T  (replicated across 128 partitions by all-ones lhsT)
  outT[h]   = sum_kc V[kc,h]^T-block @ A^T
  attnT[h]  = outT[h] * recip(sums)
  out^T     = sum_h WuT[h]^T @ attnT[h] + corr + bu

Degenerate softmax rows (all keys masked / no causal-visible key) are fixed
up exactly via two per-head mean-of-V vectors folded through the output
projection as a rank-2 correction (host-computed 0/1 row selectors).
Matmuls run as float32r (full PE rate at N>=256, near-fp32 precision).
"""

import numpy as np
import ml_dtypes

import concourse.bacc as bacc
import concourse.mybir as mybir
import concourse.tile as tile
from concourse.bass_utils import run_bass_kernel_spmd

F32 = mybir.dt.float32
F32R = mybir.dt.float32r
BF16 = mybir.dt.bfloat16

B, TQ, TK, E, H = 8, 1024, 1024, 128, 8
HE = H * E
SCALE = float(E) ** -0.5
NEG = -1.0e30


def _chunks(kc):
    """Absolute column ranges for score row kc, split at the 512 PSUM bank."""
    lo = kc * 128
    if lo < 512:
        return [(lo, 512), (512, TQ)]
    return [(lo, TQ)]


def _build():
    nc = bacc.Bacc("TRN2", target_bir_lowering=False, debug=False)
    dp = nc.declare_dram_parameter
    d_qT = dp("qT", [E, TQ], F32R, isOutput=False)
    d_kT = dp("kT", [E, TK], F32R, isOutput=False)
    d_wqT = dp("wqT", [E, HE], F32R, isOutput=False)
    d_wkT = dp("wkT", [E, HE], F32R, isOutput=False)
    d_wvT = dp("wvT", [E, HE], F32R, isOutput=False)
    d_wuT = dp("wuT", [HE, E], F32R, isOutput=False)
    d_mkb = dp("mkbias", [128, 8], F32, isOutput=False)
    d_tri = dp("trineg", [128, 128], BF16, isOutput=False)
    d_idb = dp("identb", [128, 128], BF16, isOutput=False)
    d_case = dp("caserow", [1, TQ], F32R, isOutput=False)
    d_brow = dp("brows", [2, TQ], F32R, isOutput=False)
    d_wvec = dp("wvecs", [TK, 2], F32R, isOutput=False)
    d_ones1 = dp("onesk1", [1, 128], F32R, isOutput=False)
    d_ones = dp("ones128", [128, 128], F32R, isOutput=False)
    d_bu = dp("bu", [E, 1], F32, isOutput=False)
    d_out = dp("out", [E, TQ], F32, isOutput=True)

    Exp = mybir.ActivationFunctionType.Exp
    Ident = mybir.ActivationFunctionType.Identity
    mult = mybir.AluOpType.mult
    mm = nc.tensor.matmul

    with tile.TileContext(nc) as tc:
        with (
            tc.tile_pool(name="const", bufs=1) as cp,
            tc.tile_pool(name="persist", bufs=1) as pp,
        ):
            # ---- constants ----
            wu = []
            for h in range(H):
                t = cp.tile([128, 128], F32R, tag=f"wu{h}", name=f"wu{h}")
                nc.sync.dma_start(out=t[:], in_=d_wuT[h * 128 : (h + 1) * 128, :])
                wu.append(t)
            wvec = []
            for kc in range(8):
                t = cp.tile([128, 2], F32R, tag=f"wv{kc}", name=f"wvec{kc}")
                nc.sync.dma_start(out=t[:], in_=d_wvec[kc * 128 : (kc + 1) * 128, :])
                wvec.append(t)
            mkb = cp.tile([128, 8], F32, tag="mkb", name="mkb")
            nc.sync.dma_start(out=mkb[:], in_=d_mkb[:])
            tri = cp.tile([128, 128], BF16, tag="tri", name="tri")
            nc.sync.dma_start(out=tri[:], in_=d_tri[:])
            idb = cp.tile([128, 128], BF16, tag="idb", name="idb")
            nc.sync.dma_start(out=idb[:], in_=d_idb[:])
            case = cp.tile([1, TQ], F32R, tag="case", name="case")
            nc.sync.dma_start(out=case[:], in_=d_case[:])
            brow = cp.tile([2, TQ], F32R, tag="brow", name="brow")
            nc.sync.dma_start(out=brow[:], in_=d_brow[:])
            ones1 = cp.tile([1, 128], F32R, tag="ones1", name="ones1")
            nc.sync.dma_start(out=ones1[:], in_=d_ones1[:])
            ones = cp.tile([128, 128], F32R, tag="ones", name="ones")
            nc.sync.dma_start(out=ones[:], in_=d_ones[:])
            bu = cp.tile([E, 1], F32, tag="bu", name="bu")
            nc.sync.dma_start(out=bu[:], in_=d_bu[:])

            # ---- persistent activations ----
            QT = [pp.tile([128, TQ], F32R, tag=f"QT{h}", name=f"QT{h}") for h in range(H)]
            KT = [pp.tile([128, TK], F32R, tag=f"KT{h}", name=f"KT{h}") for h in range(H)]
            V = [pp.tile([128, HE], F32R, tag=f"V{kc}", name=f"V{kc}") for kc in range(8)]
            attnT = [
                pp.tile([128, TQ], F32R, tag=f"attnT{h}", name=f"attnT{h}")
                for h in range(H)
            ]
            uT = [pp.tile([128, 2], F32R, tag=f"uT{h}", name=f"uT{h}") for h in range(H)]
            w2 = pp.tile([2, 128], F32R, tag="w2", name="w2")

            # ---- phase 1: projections ----
            with (
                tc.tile_pool(name="proj", bufs=1) as jp,
                tc.tile_pool(name="ppsum", bufs=3, space="PSUM") as jps,
            ):
                qTs = jp.tile([E, TQ], F32R, tag="qTs", name="qTs")
                nc.sync.dma_start(out=qTs[:], in_=d_qT[:])
                kTs = jp.tile([E, TK], F32R, tag="kTs", name="kTs")
                nc.sync.dma_start(out=kTs[:], in_=d_kT[:])
                wq = jp.tile([E, HE], F32R, tag="wq", name="wq")
                nc.sync.dma_start(out=wq[:], in_=d_wqT[:])
                wk = jp.tile([E, HE], F32R, tag="wk", name="wk")
                nc.sync.dma_start(out=wk[:], in_=d_wkT[:])
                wv = jp.tile([E, HE], F32R, tag="wv", name="wv")
                nc.sync.dma_start(out=wv[:], in_=d_wvT[:])

                n_evac = 0

                def evac(dst, src):
                    nonlocal n_evac
                    if n_evac % 2 == 0:
                        nc.vector.tensor_copy(dst, src)
                    else:
                        nc.scalar.copy(dst, src)
                    n_evac += 1

                for h in range(H):
                    ps = jps.tile([128, TQ], F32, tag="pps", name=f"psq{h}")
                    for a, b in ((0, 512), (512, TQ)):
                        mm(ps[:, a:b], wq[:, h * 128 : (h + 1) * 128],
                           qTs[:, a:b], start=True, stop=True)
                    evac(QT[h][:], ps[:])
                for h in range(H):
                    ps = jps.tile([128, TK], F32, tag="pps", name=f"psk{h}")
                    for a, b in ((0, 512), (512, TK)):
                        mm(ps[:, a:b], wk[:, h * 128 : (h + 1) * 128],
                           kTs[:, a:b], start=True, stop=True)
                    evac(KT[h][:], ps[:])
                for kc in range(8):
                    ps = jps.tile([128, HE], F32, tag="pps", name=f"psv{kc}")
                    for a, b in ((0, 512), (512, HE)):
                        mm(ps[:, a:b], kTs[:, kc * 128 : (kc + 1) * 128],
                           wv[:, a:b], start=True, stop=True)
                    evac(V[kc][:], ps[:])


            # ---- phase 2: degenerate-row fixup vectors ----
            with tc.tile_pool(name="fixps", bufs=2, space="PSUM") as fps:
                for h in range(H):
                    ups = fps.tile([128, 2], F32, tag="ups", name=f"ups{h}")
                    for kc in range(8):
                        mm(ups[:], V[kc][:, h * 128 : (h + 1) * 128],
                           wvec[kc][:], start=(kc == 0), stop=(kc == 7))
                    nc.vector.tensor_copy(uT[h][:], ups[:])
                w2ps = fps.tile([2, 128], F32, tag="w2ps", name="w2ps")
                for h in range(H):
                    mm(w2ps[:], uT[h][:], wu[h][:],
                       start=(h == 0), stop=(h == H - 1))
                nc.vector.tensor_copy(w2[:], w2ps[:])

            # ---- phase 3: attention per head ----
            with (
                tc.tile_pool(name="stps", bufs=2, space="PSUM") as sp,
                tc.tile_pool(name="accps", bufs=1, space="PSUM") as ap_,
                tc.tile_pool(name="atp", bufs=3) as atp,
                tc.tile_pool(name="rbp", bufs=2) as rbp,
            ):
                for h in range(H):
                    sum_ps = ap_.tile([128, TQ], F32, tag="sum_ps", name=f"sum{h}")
                    out_ps = ap_.tile([128, TQ], F32, tag="out_ps", name=f"out{h}")
                    for kc in range(8):
                        lo = kc * 128
                        ncols = TQ - lo
                        ch = _chunks(kc)
                        st = sp.tile([128, TQ], F32, tag="st", name=f"st{h}_{kc}")
                        for i, (a, b) in enumerate(ch):
                            mm(st[:, a:b], KT[h][:, lo : lo + 128],
                               QT[h][:, a:b], start=True, stop=(i == 1))
                        mm(st[:, lo : lo + 128], idb[:], tri[:],
                           start=False, stop=True)
                        at = atp.tile([128, TQ], F32R, tag="at", name=f"at{h}_{kc}")
                        nc.scalar.activation(
                            out=at[:, 0:ncols], in_=st[:, lo:TQ], func=Exp,
                            bias=mkb[:, kc : kc + 1], scale=SCALE,
                        )
                        for a, b in ch:
                            ra, rb_ = a - lo, b - lo
                            mm(sum_ps[:, a:b], ones[:], at[:, ra:rb_],
                               start=(kc == 0), stop=False)
                            stop_pv = (kc == 3 and a < 512) or kc == 7
                            mm(out_ps[:, a:b],
                               V[kc][:, h * 128 : (h + 1) * 128],
                               at[:, ra:rb_], start=(kc == 0), stop=stop_pv)
                    for a, b in ((0, 512), (512, TQ)):
                        mm(sum_ps[:, a:b], ones1[:], case[:, a:b],
                           start=False, stop=True)
                    rb = rbp.tile([128, TQ], F32, tag="rb", name=f"rb{h}")
                    nc.vector.reciprocal(out=rb[:], in_=sum_ps[:])
                    nc.vector.tensor_tensor(
                        out=attnT[h][:], in0=out_ps[:], in1=rb[:], op=mult
                    )

            # ---- phase 4: output projection ----
            with tc.tile_pool(name="finps", bufs=1, space="PSUM") as fp:
                fin = fp.tile([128, TQ], F32, tag="fin", name="fin")
                for h in range(H):
                    for a, b in ((0, 512), (512, TQ)):
                        mm(fin[:, a:b], wu[h][:], attnT[h][:, a:b],
                           start=(h == 0), stop=False)
                for a, b in ((0, 512), (512, TQ)):
                    mm(fin[:, a:b], w2[:], brow[:, a:b],
                       start=False, stop=True)
                outsb = pp.tile([E, TQ], F32, tag="outsb", name="outsb")
                nc.scalar.activation(
                    out=outsb[:], in_=fin[:], func=Ident, bias=bu[:, 0:1], scale=1.0
                )
                nc.sync.dma_start(out=d_out[:], in_=outsb[:])

    nc.compile()
    return nc


_NC = None


def _get_nc():
    global _NC
    if _NC is None:
        _NC = _build()
    return _NC


def _host_prep(q, k, mask_q, mask_k, Wq, Wk, Wv, Wu, bu):
    shared = {
        "wqT": np.ascontiguousarray(Wq.T),
        "wkT": np.ascontiguousarray(Wk.T),
        "wvT": np.ascontiguousarray(Wv.T),
        "wuT": np.ascontiguousarray(Wu.T),
        "trineg": (NEG * np.tril(np.ones((128, 128), np.float32), -1)).astype(
            ml_dtypes.bfloat16
        ),
        "identb": np.eye(128, dtype=ml_dtypes.bfloat16),
        "onesk1": np.ones((1, 128), np.float32),
        "ones128": np.ones((128, 128), np.float32),
        "bu": np.ascontiguousarray(bu[:, None]),
    }
    in_maps = []
    for b in range(B):
        mq = mask_q[b, :, 0].astype(np.float32)
        mk = mask_k[b, :, 0].astype(np.float32)
        c01 = (np.cumsum(mk) >= 1.0).astype(np.float32)
        caseA = mq * c01
        b1 = mq * (1.0 - c01)
        b2 = 1.0 - mq
        s1m = 1.0 - mk
        denom = max(float(s1m.sum()), 1.0)
        wvecs = np.stack([s1m / denom, np.full(TK, 1.0 / TK, np.float32)], axis=1)
        m = dict(shared)
        m["qT"] = np.ascontiguousarray(q[b].T)
        m["kT"] = np.ascontiguousarray(k[b].T)
        m["mkbias"] = np.ascontiguousarray(
            ((mk - 1.0) * -NEG).reshape(8, 128).T
        ).astype(np.float32)
        m["caserow"] = ((1.0 - caseA) * -NEG)[None, :].astype(np.float32)
        m["brows"] = np.stack([b1, b2]).astype(np.float32)
        m["wvecs"] = wvecs.astype(np.float32)
        in_maps.append(m)
    return in_maps


def kernel(q, k, mask_q, mask_k, Wq, Wk, Wv, Wu, bu):
    nc = _get_nc()
    in_maps = _host_prep(q, k, mask_q, mask_k, Wq, Wk, Wv, Wu, bu)
    res = run_bass_kernel_spmd(nc, in_maps, list(range(B)))
    out = np.stack([np.ascontiguousarray(res.results[b]["out"].T) for b in range(B)])
    return out.astype(np.float32)


# revision 5
# speedup vs baseline: 1.3106x; 1.3106x over previous
"""Multi-head causal+padded attention on 8 TRN2 NeuronCores.

Strategy: data-parallel over batch (8 batches -> 8 cores, no collectives).
Per core, everything is computed in a transposed layout so that no PE
transposes of the attention matrix are needed:

  QT[h] = (q Wq^T)^T slice  [e=128, tq]     KT[h] likewise
  V[kc] = (k Wv^T) row-chunk [tk=128, he]   (natural layout)
  S^T(kc,:) = KT[h][:,kc]^T-block matmuls   [tk-part, tq-free]
  A^T = exp(s * S^T + mk_bias)              (pad mask folded into exp bias,
                                             causal diag masked by a bf16
                                             identity x (-1e30 tri) matmul
                                             injected into PSUM)
  row sums  = ones^T @ A^T  (replicated across 128 partitions by all-ones lhsT)
  outT[h]   = sum_kc V[kc,h]^T-block @ A^T
  attnT[h]  = outT[h] * recip(sums)
  out^T     = sum_h WuT[h]^T @ attnT[h] + corr + bu

Degenerate softmax rows (all keys masked / no causal-visible key) are fixed
up exactly via two per-head mean-of-V vectors folded through the output
projection as a rank-2 correction (host-computed 0/1 row selectors).
Matmuls run as float32r (full PE rate at N>=256, near-fp32 precision).
"""

import numpy as np
import ml_dtypes

import concourse.bacc as bacc
import concourse.mybir as mybir
import concourse.tile as tile
from concourse.bass_utils import run_bass_kernel_spmd

F32 = mybir.dt.float32
F32R = mybir.dt.float32r
BF16 = mybir.dt.bfloat16

B, TQ, TK, E, H = 8, 1024, 1024, 128, 8
HE = H * E
SCALE = float(E) ** -0.5
NEG = -1.0e30


def _chunks(kc):
    """Absolute column ranges for score row kc, split at the 512 PSUM bank."""
    lo = kc * 128
    if lo < 512:
        return [(lo, 512), (512, TQ)]
    return [(lo, TQ)]


def _build():
    nc = bacc.Bacc("TRN2", target_bir_lowering=False, debug=False)
    dp = nc.declare_dram_parameter
    d_qT = dp("qT", [E, TQ], F32R, isOutput=False)
    d_kT = dp("kT", [E, TK], F32R, isOutput=False)
    d_wqT = dp("wqT", [E, HE], F32R, isOutput=False)
    d_wkT = dp("wkT", [E, HE], F32R, isOutput=False)
    d_wvT = dp("wvT", [E, HE], F32R, isOutput=False)
    d_wuT = dp("wuT", [HE, E], F32R, isOutput=False)
    d_mkb = dp("mkbias", [128, 8], F32, isOutput=False)
    d_tri = dp("trineg", [128, 128], BF16, isOutput=False)
    d_idb = dp("identb", [128, 128], BF16, isOutput=False)
    d_case = dp("caserow", [1, TQ], F32R, isOutput=False)
    d_brow = dp("brows", [2, TQ], F32R, isOutput=False)
    d_w2 = dp("w2", [2, E], F32R, isOutput=False)
    d_ones1 = dp("onesk1", [1, 128], F32R, isOutput=False)
    d_ones = dp("ones128", [128, 128], F32R, isOutput=False)
    d_bu = dp("bu", [E, 1], F32, isOutput=False)
    d_out = dp("out", [E, TQ], F32, isOutput=True)

    Exp = mybir.ActivationFunctionType.Exp
    Ident = mybir.ActivationFunctionType.Identity
    mult = mybir.AluOpType.mult
    mm = nc.tensor.matmul

    with tile.TileContext(nc) as tc:
        with (
            tc.tile_pool(name="const", bufs=1) as cp,
            tc.tile_pool(name="persist", bufs=1) as pp,
        ):
            # ---- constants ----
            wu = []
            for h in range(H):
                t = cp.tile([128, 128], F32R, tag=f"wu{h}", name=f"wu{h}")
                nc.sync.dma_start(out=t[:], in_=d_wuT[h * 128 : (h + 1) * 128, :])
                wu.append(t)
            mkb = cp.tile([128, 8], F32, tag="mkb", name="mkb")
            nc.sync.dma_start(out=mkb[:], in_=d_mkb[:])
            tri = cp.tile([128, 128], BF16, tag="tri", name="tri")
            nc.sync.dma_start(out=tri[:], in_=d_tri[:])
            idb = cp.tile([128, 128], BF16, tag="idb", name="idb")
            nc.sync.dma_start(out=idb[:], in_=d_idb[:])
            case = cp.tile([1, TQ], F32R, tag="case", name="case")
            nc.sync.dma_start(out=case[:], in_=d_case[:])
            brow = cp.tile([2, TQ], F32R, tag="brow", name="brow")
            nc.sync.dma_start(out=brow[:], in_=d_brow[:])
            ones1 = cp.tile([1, 128], F32R, tag="ones1", name="ones1")
            nc.sync.dma_start(out=ones1[:], in_=d_ones1[:])
            ones = cp.tile([128, 128], F32R, tag="ones", name="ones")
            nc.sync.dma_start(out=ones[:], in_=d_ones[:])
            bu = cp.tile([E, 1], F32, tag="bu", name="bu")
            nc.sync.dma_start(out=bu[:], in_=d_bu[:])
            w2 = cp.tile([2, 128], F32R, tag="w2", name="w2")
            nc.sync.dma_start(out=w2[:], in_=d_w2[:])

            # ---- persistent activations ----
            QT = [pp.tile([128, TQ], F32R, tag=f"QT{h}", name=f"QT{h}") for h in range(H)]
            KT = [pp.tile([128, TK], F32R, tag=f"KT{h}", name=f"KT{h}") for h in range(H)]
            V = [pp.tile([128, HE], F32R, tag=f"V{kc}", name=f"V{kc}") for kc in range(8)]
            attnT = [
                pp.tile([128, TQ], F32R, tag=f"attnT{h}", name=f"attnT{h}")
                for h in range(H)
            ]

            # ---- phase 1: projections ----
            with (
                tc.tile_pool(name="proj", bufs=1) as jp,
                tc.tile_pool(name="ppsum", bufs=3, space="PSUM") as jps,
            ):
                qTs = jp.tile([E, TQ], F32R, tag="qTs", name="qTs")
                nc.sync.dma_start(out=qTs[:], in_=d_qT[:])
                kTs = jp.tile([E, TK], F32R, tag="kTs", name="kTs")
                nc.sync.dma_start(out=kTs[:], in_=d_kT[:])
                wq = jp.tile([E, HE], F32R, tag="wq", name="wq")
                nc.sync.dma_start(out=wq[:], in_=d_wqT[:])
                wk = jp.tile([E, HE], F32R, tag="wk", name="wk")
                nc.sync.dma_start(out=wk[:], in_=d_wkT[:])
                wv = jp.tile([E, HE], F32R, tag="wv", name="wv")
                nc.sync.dma_start(out=wv[:], in_=d_wvT[:])

                n_evac = 0

                def evac(dst, src):
                    nonlocal n_evac
                    if n_evac % 2 == 0:
                        nc.vector.tensor_copy(dst, src)
                    else:
                        nc.scalar.copy(dst, src)
                    n_evac += 1

                for h in range(H):
                    ps = jps.tile([128, TQ], F32, tag="pps", name=f"psq{h}")
                    for a, b in ((0, 512), (512, TQ)):
                        mm(ps[:, a:b], wq[:, h * 128 : (h + 1) * 128],
                           qTs[:, a:b], start=True, stop=True)
                    evac(QT[h][:], ps[:])
                for h in range(H):
                    ps = jps.tile([128, TK], F32, tag="pps", name=f"psk{h}")
                    for a, b in ((0, 512), (512, TK)):
                        mm(ps[:, a:b], wk[:, h * 128 : (h + 1) * 128],
                           kTs[:, a:b], start=True, stop=True)
                    evac(KT[h][:], ps[:])
                for kc in range(8):
                    ps = jps.tile([128, HE], F32, tag="pps", name=f"psv{kc}")
                    for a, b in ((0, 512), (512, HE)):
                        mm(ps[:, a:b], kTs[:, kc * 128 : (kc + 1) * 128],
                           wv[:, a:b], start=True, stop=True)
                    evac(V[kc][:], ps[:])


            # ---- phase 3: attention per head ----
            with (
                tc.tile_pool(name="stps", bufs=2, space="PSUM") as sp,
                tc.tile_pool(name="accps", bufs=1, space="PSUM") as ap_,
                tc.tile_pool(name="atp", bufs=3) as atp,
                tc.tile_pool(name="rbp", bufs=2) as rbp,
            ):
                for h in range(H):
                    sum_ps = ap_.tile([128, TQ], F32, tag="sum_ps", name=f"sum{h}")
                    out_ps = ap_.tile([128, TQ], F32, tag="out_ps", name=f"out{h}")
                    for kc in range(8):
                        lo = kc * 128
                        ncols = TQ - lo
                        ch = _chunks(kc)
                        st = sp.tile([128, TQ], F32, tag="st", name=f"st{h}_{kc}")
                        for i, (a, b) in enumerate(ch):
                            mm(st[:, a:b], KT[h][:, lo : lo + 128],
                               QT[h][:, a:b], start=True, stop=(i == 1))
                        mm(st[:, lo : lo + 128], idb[:], tri[:],
                           start=False, stop=True)
                        at = atp.tile([128, TQ], F32R, tag="at", name=f"at{h}_{kc}")
                        nc.scalar.activation(
                            out=at[:, 0:ncols], in_=st[:, lo:TQ], func=Exp,
                            bias=mkb[:, kc : kc + 1], scale=SCALE,
                        )
                        for a, b in ch:
                            ra, rb_ = a - lo, b - lo
                            mm(sum_ps[:, a:b], ones[:], at[:, ra:rb_],
                               start=(kc == 0), stop=False)
                            stop_pv = (kc == 3 and a < 512) or kc == 7
                            mm(out_ps[:, a:b],
                               V[kc][:, h * 128 : (h + 1) * 128],
                               at[:, ra:rb_], start=(kc == 0), stop=stop_pv)
                    for a, b in ((0, 512), (512, TQ)):
                        mm(sum_ps[:, a:b], ones1[:], case[:, a:b],
                           start=False, stop=True)
                    sum_sb = rbp.tile([128, TQ], F32, tag="sum_sb", name=f"ssb{h}")
                    nc.scalar.copy(sum_sb[:], sum_ps[:])
                    out_sb = rbp.tile([128, TQ], F32, tag="out_sb", name=f"osb{h}")
                    nc.vector.tensor_copy(out_sb[:], out_ps[:])
                    rb = rbp.tile([128, TQ], F32, tag="rb", name=f"rb{h}")
                    nc.vector.reciprocal(out=rb[:], in_=sum_sb[:])
                    nc.vector.tensor_tensor(
                        out=attnT[h][:], in0=out_sb[:], in1=rb[:], op=mult
                    )

            # ---- phase 4: output projection ----
            with tc.tile_pool(name="finps", bufs=1, space="PSUM") as fp:
                fin = fp.tile([128, TQ], F32, tag="fin", name="fin")
                for h in range(H):
                    for a, b in ((0, 512), (512, TQ)):
                        mm(fin[:, a:b], wu[h][:], attnT[h][:, a:b],
                           start=(h == 0), stop=False)
                for a, b in ((0, 512), (512, TQ)):
                    mm(fin[:, a:b], w2[:], brow[:, a:b],
                       start=False, stop=True)
                outsb = pp.tile([E, TQ], F32, tag="outsb", name="outsb")
                nc.scalar.activation(
                    out=outsb[:], in_=fin[:], func=Ident, bias=bu[:, 0:1], scale=1.0
                )
                nc.sync.dma_start(out=d_out[:], in_=outsb[:])

    nc.compile()
    return nc


_NC = None


def _get_nc():
    global _NC
    if _NC is None:
        _NC = _build()
    return _NC


def _host_prep(q, k, mask_q, mask_k, Wq, Wk, Wv, Wu, bu):
    shared = {
        "wqT": np.ascontiguousarray(Wq.T),
        "wkT": np.ascontiguousarray(Wk.T),
        "wvT": np.ascontiguousarray(Wv.T),
        "wuT": np.ascontiguousarray(Wu.T),
        "trineg": (NEG * np.tril(np.ones((128, 128), np.float32), -1)).astype(
            ml_dtypes.bfloat16
        ),
        "identb": np.eye(128, dtype=ml_dtypes.bfloat16),
        "onesk1": np.ones((1, 128), np.float32),
        "ones128": np.ones((128, 128), np.float32),
        "bu": np.ascontiguousarray(bu[:, None]),
    }
    WuWv = (Wu @ Wv).astype(np.float32)
    in_maps = []
    for b in range(B):
        mq = mask_q[b, :, 0].astype(np.float32)
        mk = mask_k[b, :, 0].astype(np.float32)
        c01 = (np.cumsum(mk) >= 1.0).astype(np.float32)
        caseA = mq * c01
        b1 = mq * (1.0 - c01)
        b2 = 1.0 - mq
        s1m = 1.0 - mk
        denom = max(float(s1m.sum()), 1.0)
        wvecs = np.stack([s1m / denom, np.full(TK, 1.0 / TK, np.float32)], axis=1)
        w2 = (wvecs.T.astype(np.float32) @ k[b]) @ WuWv.T
        m = dict(shared)
        m["qT"] = np.ascontiguousarray(q[b].T)
        m["kT"] = np.ascontiguousarray(k[b].T)
        m["mkbias"] = np.ascontiguousarray(
            ((mk - 1.0) * -NEG).reshape(8, 128).T
        ).astype(np.float32)
        m["caserow"] = ((1.0 - caseA) * -NEG)[None, :].astype(np.float32)
        m["brows"] = np.stack([b1, b2]).astype(np.float32)
        m["w2"] = np.ascontiguousarray(w2.astype(np.float32))
        in_maps.append(m)
    return in_maps


def kernel(q, k, mask_q, mask_k, Wq, Wk, Wv, Wu, bu):
    nc = _get_nc()
    in_maps = _host_prep(q, k, mask_q, mask_k, Wq, Wk, Wv, Wu, bu)
    res = run_bass_kernel_spmd(nc, in_maps, list(range(B)))
    out = np.stack([np.ascontiguousarray(res.results[b]["out"].T) for b in range(B)])
    return out.astype(np.float32)


# revision 6
# speedup vs baseline: 1.4064x; 1.0731x over previous
"""Multi-head causal+padded attention on 8 TRN2 NeuronCores.

Strategy: data-parallel over batch (8 batches -> 8 cores, no collectives).
Per core, everything is computed in a transposed layout so that no PE
transposes of the attention matrix are needed:

  QT[h] = (q Wq^T)^T slice  [e=128, tq]     KT[h] likewise
  V[kc] = (k Wv^T) row-chunk [tk=128, he]   (natural layout)
  S^T(kc,:) = KT[h][:,kc]^T-block matmuls   [tk-part, tq-free]
  A^T = exp(s * S^T + mk_bias)              (pad mask folded into exp bias,
                                             causal diag masked by a bf16
                                             identity x (-1e30 tri) matmul
                                             injected into PSUM)
  row sums  = ones^T @ A^T  (replicated across 128 partitions by all-ones lhsT)
  outT[h]   = sum_kc V[kc,h]^T-block @ A^T
  attnT[h]  = outT[h] * recip(sums)
  out^T     = sum_h WuT[h]^T @ attnT[h] + corr + bu

Degenerate softmax rows (all keys masked / no causal-visible key) are fixed
up exactly via two per-head mean-of-V vectors folded through the output
projection as a rank-2 correction (host-computed 0/1 row selectors).
Matmuls run as float32r (full PE rate at N>=256, near-fp32 precision).
"""

import numpy as np
import ml_dtypes

import concourse.bacc as bacc
import concourse.mybir as mybir
import concourse.tile as tile
from concourse.bass_utils import run_bass_kernel_spmd

F32 = mybir.dt.float32
F32R = mybir.dt.float32r
BF16 = mybir.dt.bfloat16
F16 = mybir.dt.float16

import os
SCORE_DT = {"f32r": F32R, "f16": F16}[os.environ.get("K_SCORE_DT", "f32r")]
SOFT_DT = {"f32r": F32R, "f16": F16}[os.environ.get("K_SOFT_DT", "f16")]
TRI_NEG = -60000.0 if SOFT_DT == F16 else -1.0e30
_SOFT_NP = {F16: "float16", F32R: "float32"}

B, TQ, TK, E, H = 8, 1024, 1024, 128, 8
HE = H * E
SCALE = float(E) ** -0.5
NEG = -1.0e30


def _chunks(kc):
    """Absolute column ranges for score row kc, split at the 512 PSUM bank."""
    lo = kc * 128
    if lo < 512:
        return [(lo, 512), (512, TQ)]
    return [(lo, TQ)]


def _build():
    nc = bacc.Bacc("TRN2", target_bir_lowering=False, debug=False)
    dp = nc.declare_dram_parameter
    d_qT = dp("qT", [E, TQ], F32R, isOutput=False)
    d_kT = dp("kT", [E, TK], F32R, isOutput=False)
    d_wqT = dp("wqT", [E, HE], F32R, isOutput=False)
    d_wkT = dp("wkT", [E, HE], F32R, isOutput=False)
    d_wvT = dp("wvT", [E, HE], F32R, isOutput=False)
    d_wuT = dp("wuT", [HE, E], SOFT_DT, isOutput=False)
    d_mkb = dp("mkbias", [128, 8], F32, isOutput=False)
    d_tri = dp("trineg", [128, 128], SOFT_DT, isOutput=False)
    d_idb = dp("identb", [128, 128], SOFT_DT, isOutput=False)
    d_case = dp("caserow", [1, TQ], F32R, isOutput=False)
    d_brow = dp("brows", [2, TQ], F32R, isOutput=False)
    d_w2 = dp("w2", [2, E], F32R, isOutput=False)
    d_ones1 = dp("onesk1", [1, 128], F32R, isOutput=False)
    d_ones = dp("ones128", [128, 128], SOFT_DT, isOutput=False)
    d_bu = dp("bu", [E, 1], F32, isOutput=False)
    d_out = dp("out", [E, TQ], F32, isOutput=True)

    Exp = mybir.ActivationFunctionType.Exp
    Ident = mybir.ActivationFunctionType.Identity
    mult = mybir.AluOpType.mult
    mm = nc.tensor.matmul

    with tile.TileContext(nc) as tc:
        with (
            tc.tile_pool(name="const", bufs=1) as cp,
            tc.tile_pool(name="persist", bufs=1) as pp,
        ):
            # ---- constants ----
            wu = []
            for h in range(H):
                t = cp.tile([128, 128], SOFT_DT, tag=f"wu{h}", name=f"wu{h}")
                nc.sync.dma_start(out=t[:], in_=d_wuT[h * 128 : (h + 1) * 128, :])
                wu.append(t)
            mkb = cp.tile([128, 8], F32, tag="mkb", name="mkb")
            nc.sync.dma_start(out=mkb[:], in_=d_mkb[:])
            tri = cp.tile([128, 128], SOFT_DT, tag="tri", name="tri")
            nc.sync.dma_start(out=tri[:], in_=d_tri[:])
            idb = cp.tile([128, 128], SOFT_DT, tag="idb", name="idb")
            nc.sync.dma_start(out=idb[:], in_=d_idb[:])
            case = cp.tile([1, TQ], F32R, tag="case", name="case")
            nc.sync.dma_start(out=case[:], in_=d_case[:])
            brow = cp.tile([2, TQ], F32R, tag="brow", name="brow")
            nc.sync.dma_start(out=brow[:], in_=d_brow[:])
            ones1 = cp.tile([1, 128], F32R, tag="ones1", name="ones1")
            nc.sync.dma_start(out=ones1[:], in_=d_ones1[:])
            ones = cp.tile([128, 128], SOFT_DT, tag="ones", name="ones")
            nc.sync.dma_start(out=ones[:], in_=d_ones[:])
            bu = cp.tile([E, 1], F32, tag="bu", name="bu")
            nc.sync.dma_start(out=bu[:], in_=d_bu[:])
            w2 = cp.tile([2, 128], F32R, tag="w2", name="w2")
            nc.sync.dma_start(out=w2[:], in_=d_w2[:])

            # ---- persistent activations ----
            QT = [pp.tile([128, TQ], SCORE_DT, tag=f"QT{h}", name=f"QT{h}") for h in range(H)]
            KT = [pp.tile([128, TK], SCORE_DT, tag=f"KT{h}", name=f"KT{h}") for h in range(H)]
            V = [pp.tile([128, HE], SOFT_DT, tag=f"V{kc}", name=f"V{kc}") for kc in range(8)]
            attnT = [
                pp.tile([128, TQ], SOFT_DT, tag=f"attnT{h}", name=f"attnT{h}")
                for h in range(H)
            ]

            # ---- phase 1: projections ----
            with (
                tc.tile_pool(name="proj", bufs=1) as jp,
                tc.tile_pool(name="ppsum", bufs=3, space="PSUM") as jps,
            ):
                qTs = jp.tile([E, TQ], F32R, tag="qTs", name="qTs")
                nc.sync.dma_start(out=qTs[:], in_=d_qT[:])
                kTs = jp.tile([E, TK], F32R, tag="kTs", name="kTs")
                nc.sync.dma_start(out=kTs[:], in_=d_kT[:])
                wq = jp.tile([E, HE], F32R, tag="wq", name="wq")
                nc.sync.dma_start(out=wq[:], in_=d_wqT[:])
                wk = jp.tile([E, HE], F32R, tag="wk", name="wk")
                nc.sync.dma_start(out=wk[:], in_=d_wkT[:])
                wv = jp.tile([E, HE], F32R, tag="wv", name="wv")
                nc.sync.dma_start(out=wv[:], in_=d_wvT[:])

                n_evac = 0

                def evac(dst, src):
                    nonlocal n_evac
                    if n_evac % 2 == 0:
                        nc.vector.tensor_copy(dst, src)
                    else:
                        nc.scalar.copy(dst, src)
                    n_evac += 1

                for h in range(H):
                    ps = jps.tile([128, TQ], F32, tag="pps", name=f"psq{h}")
                    for a, b in ((0, 512), (512, TQ)):
                        mm(ps[:, a:b], wq[:, h * 128 : (h + 1) * 128],
                           qTs[:, a:b], start=True, stop=True)
                    evac(QT[h][:], ps[:])
                for h in range(H):
                    ps = jps.tile([128, TK], F32, tag="pps", name=f"psk{h}")
                    for a, b in ((0, 512), (512, TK)):
                        mm(ps[:, a:b], wk[:, h * 128 : (h + 1) * 128],
                           kTs[:, a:b], start=True, stop=True)
                    evac(KT[h][:], ps[:])
                for kc in range(8):
                    ps = jps.tile([128, HE], F32, tag="pps", name=f"psv{kc}")
                    for a, b in ((0, 512), (512, HE)):
                        mm(ps[:, a:b], kTs[:, kc * 128 : (kc + 1) * 128],
                           wv[:, a:b], start=True, stop=True)
                    evac(V[kc][:], ps[:])


            # ---- phase 3: attention per head ----
            with (
                tc.tile_pool(name="stps", bufs=2, space="PSUM") as sp,
                tc.tile_pool(name="accps", bufs=1, space="PSUM") as ap_,
                tc.tile_pool(name="atp", bufs=3) as atp,
                tc.tile_pool(name="rbp", bufs=2) as rbp,
            ):
                for h in range(H):
                    sum_ps = ap_.tile([128, TQ], F32, tag="sum_ps", name=f"sum{h}")
                    out_ps = ap_.tile([128, TQ], F32, tag="out_ps", name=f"out{h}")
                    for kc in range(8):
                        lo = kc * 128
                        ncols = TQ - lo
                        ch = _chunks(kc)
                        st = sp.tile([128, TQ], F32, tag="st", name=f"st{h}_{kc}")
                        for i, (a, b) in enumerate(ch):
                            mm(st[:, a:b], KT[h][:, lo : lo + 128],
                               QT[h][:, a:b], start=True, stop=(i == 1))
                        mm(st[:, lo : lo + 128], idb[:], tri[:],
                           start=False, stop=True)
                        at = atp.tile([128, TQ], SOFT_DT, tag="at", name=f"at{h}_{kc}")
                        nc.scalar.activation(
                            out=at[:, 0:ncols], in_=st[:, lo:TQ], func=Exp,
                            bias=mkb[:, kc : kc + 1], scale=SCALE,
                        )
                        for a, b in ch:
                            ra, rb_ = a - lo, b - lo
                            mm(sum_ps[:, a:b], ones[:], at[:, ra:rb_],
                               start=(kc == 0), stop=False)
                            stop_pv = (kc == 3 and a < 512) or kc == 7
                            mm(out_ps[:, a:b],
                               V[kc][:, h * 128 : (h + 1) * 128],
                               at[:, ra:rb_], start=(kc == 0), stop=stop_pv)
                    for a, b in ((0, 512), (512, TQ)):
                        mm(sum_ps[:, a:b], ones1[:], case[:, a:b],
                           start=False, stop=True)
                    sum_sb = rbp.tile([128, TQ], F32, tag="sum_sb", name=f"ssb{h}")
                    nc.scalar.copy(sum_sb[:], sum_ps[:])
                    out_sb = rbp.tile([128, TQ], F32, tag="out_sb", name=f"osb{h}")
                    nc.vector.tensor_copy(out_sb[:], out_ps[:])
                    rb = rbp.tile([128, TQ], F32, tag="rb", name=f"rb{h}")
                    nc.vector.reciprocal(out=rb[:], in_=sum_sb[:])
                    nc.vector.tensor_tensor(
                        out=attnT[h][:], in0=out_sb[:], in1=rb[:], op=mult
                    )

            # ---- phase 4: output projection ----
            with tc.tile_pool(name="finps", bufs=1, space="PSUM") as fp:
                fin = fp.tile([128, TQ], F32, tag="fin", name="fin")
                for h in range(H):
                    for a, b in ((0, 512), (512, TQ)):
                        mm(fin[:, a:b], wu[h][:], attnT[h][:, a:b],
                           start=(h == 0), stop=False)
                for a, b in ((0, 512), (512, TQ)):
                    mm(fin[:, a:b], w2[:], brow[:, a:b],
                       start=False, stop=True)
                outsb = pp.tile([E, TQ], F32, tag="outsb", name="outsb")
                nc.scalar.activation(
                    out=outsb[:], in_=fin[:], func=Ident, bias=bu[:, 0:1], scale=1.0
                )
                nc.sync.dma_start(out=d_out[:], in_=outsb[:])

    nc.compile()
    return nc


_NC = None


def _get_nc():
    global _NC
    if _NC is None:
        _NC = _build()
    return _NC


def _host_prep(q, k, mask_q, mask_k, Wq, Wk, Wv, Wu, bu):
    shared = {
        "wqT": np.ascontiguousarray(Wq.T),
        "wkT": np.ascontiguousarray(Wk.T),
        "wvT": np.ascontiguousarray(Wv.T),
        "wuT": np.ascontiguousarray(Wu.T).astype(_SOFT_NP[SOFT_DT]),
        "trineg": (TRI_NEG * np.tril(np.ones((128, 128), np.float32), -1)).astype(_SOFT_NP[SOFT_DT]),
        "identb": np.eye(128).astype(_SOFT_NP[SOFT_DT]),
        "onesk1": np.ones((1, 128), np.float32),
        "ones128": np.ones((128, 128)).astype(_SOFT_NP[SOFT_DT]),
        "bu": np.ascontiguousarray(bu[:, None]),
    }
    WuWv = (Wu @ Wv).astype(np.float32)
    in_maps = []
    for b in range(B):
        mq = mask_q[b, :, 0].astype(np.float32)
        mk = mask_k[b, :, 0].astype(np.float32)
        c01 = (np.cumsum(mk) >= 1.0).astype(np.float32)
        caseA = mq * c01
        b1 = mq * (1.0 - c01)
        b2 = 1.0 - mq
        s1m = 1.0 - mk
        denom = max(float(s1m.sum()), 1.0)
        wvecs = np.stack([s1m / denom, np.full(TK, 1.0 / TK, np.float32)], axis=1)
        w2 = (wvecs.T.astype(np.float32) @ k[b]) @ WuWv.T
        m = dict(shared)
        m["qT"] = np.ascontiguousarray(q[b].T)
        m["kT"] = np.ascontiguousarray(k[b].T)
        m["mkbias"] = np.ascontiguousarray(
            ((mk - 1.0) * -NEG).reshape(8, 128).T
        ).astype(np.float32)
        m["caserow"] = ((1.0 - caseA) * -NEG)[None, :].astype(np.float32)
        m["brows"] = np.stack([b1, b2]).astype(np.float32)
        m["w2"] = np.ascontiguousarray(w2.astype(np.float32))
        in_maps.append(m)
    return in_maps


def kernel(q, k, mask_q, mask_k, Wq, Wk, Wv, Wu, bu):
    nc = _get_nc()
    in_maps = _host_prep(q, k, mask_q, mask_k, Wq, Wk, Wv, Wu, bu)
    res = run_bass_kernel_spmd(nc, in_maps, list(range(B)))
    out = np.stack([np.ascontiguousarray(res.results[b]["out"].T) for b in range(B)])
    return out.astype(np.float32)


# revision 7
# speedup vs baseline: 1.5074x; 1.0719x over previous
"""Multi-head causal+padded attention on 8 TRN2 NeuronCores.

Strategy: data-parallel over batch (8 batches -> 8 cores, no collectives).
Per core, everything is computed in a transposed layout so that no PE
transposes of the attention matrix are needed:

  QT[h] = (q Wq^T)^T slice  [e=128, tq]     KT[h] likewise
  V[kc] = (k Wv^T) row-chunk [tk=128, he]   (natural layout)
  S^T(kc,:) = KT[h][:,kc]^T-block matmuls   [tk-part, tq-free]
  A^T = exp(s * S^T + mk_bias)              (pad mask folded into exp bias,
                                             causal diag masked by a bf16
                                             identity x (-1e30 tri) matmul
                                             injected into PSUM)
  row sums  = ones^T @ A^T  (replicated across 128 partitions by all-ones lhsT)
  outT[h]   = sum_kc V[kc,h]^T-block @ A^T
  attnT[h]  = outT[h] * recip(sums)
  out^T     = sum_h WuT[h]^T @ attnT[h] + corr + bu

Degenerate softmax rows (all keys masked / no causal-visible key) are fixed
up exactly via two per-head mean-of-V vectors folded through the output
projection as a rank-2 correction (host-computed 0/1 row selectors).
Matmuls run as float32r (full PE rate at N>=256, near-fp32 precision).
"""

import numpy as np
import ml_dtypes

import concourse.bacc as bacc
import concourse.mybir as mybir
import concourse.tile as tile
from concourse.bass_utils import run_bass_kernel_spmd

F32 = mybir.dt.float32
F32R = mybir.dt.float32r
BF16 = mybir.dt.bfloat16
F16 = mybir.dt.float16

import os
SCORE_DT = {"f32r": F32R, "f16": F16}[os.environ.get("K_SCORE_DT", "f32r")]
SOFT_DT = {"f32r": F32R, "f16": F16}[os.environ.get("K_SOFT_DT", "f16")]
TRI_NEG = -60000.0 if SOFT_DT == F16 else -1.0e30
_SOFT_NP = {F16: "float16", F32R: "float32"}

B, TQ, TK, E, H = 8, 1024, 1024, 128, 8
HE = H * E
SCALE = float(E) ** -0.5
NEG = -1.0e30


def _chunks(kc):
    """Absolute column ranges for score row kc, split at the 512 PSUM bank."""
    lo = kc * 128
    if lo < 512:
        return [(lo, 512), (512, TQ)]
    return [(lo, TQ)]


def _build():
    nc = bacc.Bacc("TRN2", target_bir_lowering=False, debug=False)
    dp = nc.declare_dram_parameter
    d_qT = dp("qT", [E, TQ], F32R, isOutput=False)
    d_kT = dp("kT", [E, TK], F32R, isOutput=False)
    d_wqT = dp("wqT", [E, HE], F32R, isOutput=False)
    d_wkT = dp("wkT", [E, HE], F32R, isOutput=False)
    d_wvT = dp("wvT", [E, HE], F32R, isOutput=False)
    d_wuT = dp("wuT", [HE, E], SOFT_DT, isOutput=False)
    d_mkb = dp("mkbias", [128, 8], F32, isOutput=False)
    d_tri = dp("trineg", [128, 128], SOFT_DT, isOutput=False)
    d_idb = dp("identb", [128, 128], SOFT_DT, isOutput=False)
    d_case = dp("caserow", [1, TQ], F32R, isOutput=False)
    d_brow = dp("brows", [2, TQ], F32R, isOutput=False)
    d_w2 = dp("w2", [2, E], F32R, isOutput=False)
    d_ones1 = dp("onesk1", [1, 128], F32R, isOutput=False)
    d_ones = dp("ones128", [128, 128], SOFT_DT, isOutput=False)
    d_bu = dp("bu", [E, 1], F32, isOutput=False)
    d_out = dp("out", [E, TQ], F32, isOutput=True)

    Exp = mybir.ActivationFunctionType.Exp
    Ident = mybir.ActivationFunctionType.Identity
    mult = mybir.AluOpType.mult
    mm = nc.tensor.matmul

    with tile.TileContext(nc) as tc:
        with (
            tc.tile_pool(name="const", bufs=1) as cp,
            tc.tile_pool(name="persist", bufs=1) as pp,
        ):
            # ---- constants ----
            wu = []
            for h in range(H):
                t = cp.tile([128, 128], SOFT_DT, tag=f"wu{h}", name=f"wu{h}")
                nc.sync.dma_start(out=t[:], in_=d_wuT[h * 128 : (h + 1) * 128, :])
                wu.append(t)
            mkb = cp.tile([128, 8], F32, tag="mkb", name="mkb")
            nc.sync.dma_start(out=mkb[:], in_=d_mkb[:])
            tri = cp.tile([128, 128], SOFT_DT, tag="tri", name="tri")
            nc.sync.dma_start(out=tri[:], in_=d_tri[:])
            idb = cp.tile([128, 128], SOFT_DT, tag="idb", name="idb")
            nc.sync.dma_start(out=idb[:], in_=d_idb[:])
            case = cp.tile([1, TQ], F32R, tag="case", name="case")
            nc.sync.dma_start(out=case[:], in_=d_case[:])
            brow = cp.tile([2, TQ], F32R, tag="brow", name="brow")
            nc.sync.dma_start(out=brow[:], in_=d_brow[:])
            ones1 = cp.tile([1, 128], F32R, tag="ones1", name="ones1")
            nc.sync.dma_start(out=ones1[:], in_=d_ones1[:])
            ones = cp.tile([128, 128], SOFT_DT, tag="ones", name="ones")
            nc.sync.dma_start(out=ones[:], in_=d_ones[:])
            bu = cp.tile([E, 1], F32, tag="bu", name="bu")
            nc.sync.dma_start(out=bu[:], in_=d_bu[:])
            w2 = cp.tile([2, 128], F32R, tag="w2", name="w2")
            nc.sync.dma_start(out=w2[:], in_=d_w2[:])

            # ---- persistent activations ----
            QT = [pp.tile([128, TQ], SCORE_DT, tag=f"QT{h}", name=f"QT{h}") for h in range(H)]
            KT = [pp.tile([128, TK], SCORE_DT, tag=f"KT{h}", name=f"KT{h}") for h in range(H)]
            V = [pp.tile([128, HE], SOFT_DT, tag=f"V{kc}", name=f"V{kc}") for kc in range(8)]
            attnT = [
                pp.tile([128, TQ], SOFT_DT, tag=f"attnT{h}", name=f"attnT{h}")
                for h in range(H)
            ]

            # ---- phase 1: projections ----
            with (
                tc.tile_pool(name="proj", bufs=1) as jp,
                tc.tile_pool(name="ppsum", bufs=3, space="PSUM") as jps,
            ):
                qTs = jp.tile([E, TQ], F32R, tag="qTs", name="qTs")
                nc.sync.dma_start(out=qTs[:], in_=d_qT[:])
                kTs = jp.tile([E, TK], F32R, tag="kTs", name="kTs")
                nc.sync.dma_start(out=kTs[:], in_=d_kT[:])
                wq = jp.tile([E, HE], F32R, tag="wq", name="wq")
                nc.sync.dma_start(out=wq[:], in_=d_wqT[:])
                wk = jp.tile([E, HE], F32R, tag="wk", name="wk")
                nc.sync.dma_start(out=wk[:], in_=d_wkT[:])
                wv = jp.tile([E, HE], F32R, tag="wv", name="wv")
                nc.sync.dma_start(out=wv[:], in_=d_wvT[:])

                n_evac = 0

                def evac(dst, src):
                    nonlocal n_evac
                    if n_evac % 2 == 0:
                        nc.vector.tensor_copy(dst, src)
                    else:
                        nc.scalar.copy(dst, src)
                    n_evac += 1

                for h in range(H):
                    ps = jps.tile([128, TQ], F32, tag="pps", name=f"psq{h}")
                    for a, b in ((0, 512), (512, TQ)):
                        mm(ps[:, a:b], wq[:, h * 128 : (h + 1) * 128],
                           qTs[:, a:b], start=True, stop=True)
                    evac(QT[h][:], ps[:])
                for h in range(H):
                    ps = jps.tile([128, TK], F32, tag="pps", name=f"psk{h}")
                    for a, b in ((0, 512), (512, TK)):
                        mm(ps[:, a:b], wk[:, h * 128 : (h + 1) * 128],
                           kTs[:, a:b], start=True, stop=True)
                    evac(KT[h][:], ps[:])
                for kc in range(8):
                    ps = jps.tile([128, HE], F32, tag="pps", name=f"psv{kc}")
                    for a, b in ((0, 512), (512, HE)):
                        mm(ps[:, a:b], kTs[:, kc * 128 : (kc + 1) * 128],
                           wv[:, a:b], start=True, stop=True)
                    evac(V[kc][:], ps[:])


            # ---- phase 3: attention per head ----
            with (
                tc.tile_pool(name="stps", bufs=2, space="PSUM") as sp,
                tc.tile_pool(name="accps", bufs=1, space="PSUM") as ap_,
                tc.tile_pool(name="atp", bufs=3) as atp,
                tc.tile_pool(name="rbp", bufs=2) as rbp,
            ):
                for h in range(H):
                    sum_ps = ap_.tile([128, TQ], F32, tag="sum_ps", name=f"sum{h}")
                    out_ps = ap_.tile([128, TQ], F32, tag="out_ps", name=f"out{h}")
                    ats = [None] * 8

                    def consume(kc):
                        lo = kc * 128
                        for a, b in _chunks(kc):
                            ra, rb_ = a - lo, b - lo
                            mm(sum_ps[:, a:b], ones[:], ats[kc][:, ra:rb_],
                               start=(kc == 0), stop=False)
                            stop_pv = (kc == 3 and a < 512) or kc == 7
                            mm(out_ps[:, a:b],
                               V[kc][:, h * 128 : (h + 1) * 128],
                               ats[kc][:, ra:rb_], start=(kc == 0),
                               stop=stop_pv)

                    for kc in range(8):
                        lo = kc * 128
                        ncols = TQ - lo
                        ch = _chunks(kc)
                        st = sp.tile([128, TQ], F32, tag="st", name=f"st{h}_{kc}")
                        for i, (a, b) in enumerate(ch):
                            mm(st[:, a:b], KT[h][:, lo : lo + 128],
                               QT[h][:, a:b], start=True, stop=(i == 1))
                        mm(st[:, lo : lo + 128], idb[:], tri[:],
                           start=False, stop=True)
                        at = atp.tile([128, TQ], SOFT_DT, tag="at", name=f"at{h}_{kc}")
                        ats[kc] = at
                        nc.scalar.activation(
                            out=at[:, 0:ncols], in_=st[:, lo:TQ], func=Exp,
                            bias=mkb[:, kc : kc + 1], scale=SCALE,
                        )
                        if kc >= 1:
                            consume(kc - 1)
                    consume(7)
                    for a, b in ((0, 512), (512, TQ)):
                        mm(sum_ps[:, a:b], ones1[:], case[:, a:b],
                           start=False, stop=True)
                    sum_sb = rbp.tile([128, TQ], F32, tag="sum_sb", name=f"ssb{h}")
                    nc.scalar.copy(sum_sb[:], sum_ps[:])
                    out_sb = rbp.tile([128, TQ], F32, tag="out_sb", name=f"osb{h}")
                    nc.vector.tensor_copy(out_sb[:], out_ps[:])
                    rb = rbp.tile([128, TQ], F32, tag="rb", name=f"rb{h}")
                    nc.vector.reciprocal(out=rb[:], in_=sum_sb[:])
                    nc.vector.tensor_tensor(
                        out=attnT[h][:], in0=out_sb[:], in1=rb[:], op=mult
                    )

            # ---- phase 4: output projection ----
            with tc.tile_pool(name="finps", bufs=1, space="PSUM") as fp:
                fin = fp.tile([128, TQ], F32, tag="fin", name="fin")
                for h in range(H):
                    for a, b in ((0, 512), (512, TQ)):
                        mm(fin[:, a:b], wu[h][:], attnT[h][:, a:b],
                           start=(h == 0), stop=False)
                for a, b in ((0, 512), (512, TQ)):
                    mm(fin[:, a:b], w2[:], brow[:, a:b],
                       start=False, stop=True)
                outsb = pp.tile([E, TQ], F32, tag="outsb", name="outsb")
                nc.scalar.activation(
                    out=outsb[:], in_=fin[:], func=Ident, bias=bu[:, 0:1], scale=1.0
                )
                nc.sync.dma_start(out=d_out[:], in_=outsb[:])

    nc.compile()
    return nc


_NC = None


def _get_nc():
    global _NC
    if _NC is None:
        _NC = _build()
    return _NC


def _host_prep(q, k, mask_q, mask_k, Wq, Wk, Wv, Wu, bu):
    shared = {
        "wqT": np.ascontiguousarray(Wq.T),
        "wkT": np.ascontiguousarray(Wk.T),
        "wvT": np.ascontiguousarray(Wv.T),
        "wuT": np.ascontiguousarray(Wu.T).astype(_SOFT_NP[SOFT_DT]),
        "trineg": (TRI_NEG * np.tril(np.ones((128, 128), np.float32), -1)).astype(_SOFT_NP[SOFT_DT]),
        "identb": np.eye(128).astype(_SOFT_NP[SOFT_DT]),
        "onesk1": np.ones((1, 128), np.float32),
        "ones128": np.ones((128, 128)).astype(_SOFT_NP[SOFT_DT]),
        "bu": np.ascontiguousarray(bu[:, None]),
    }
    WuWv = (Wu @ Wv).astype(np.float32)
    in_maps = []
    for b in range(B):
        mq = mask_q[b, :, 0].astype(np.float32)
        mk = mask_k[b, :, 0].astype(np.float32)
        c01 = (np.cumsum(mk) >= 1.0).astype(np.float32)
        caseA = mq * c01
        b1 = mq * (1.0 - c01)
        b2 = 1.0 - mq
        s1m = 1.0 - mk
        denom = max(float(s1m.sum()), 1.0)
        wvecs = np.stack([s1m / denom, np.full(TK, 1.0 / TK, np.float32)], axis=1)
        w2 = (wvecs.T.astype(np.float32) @ k[b]) @ WuWv.T
        m = dict(shared)
        m["qT"] = np.ascontiguousarray(q[b].T)
        m["kT"] = np.ascontiguousarray(k[b].T)
        m["mkbias"] = np.ascontiguousarray(
            ((mk - 1.0) * -NEG).reshape(8, 128).T
        ).astype(np.float32)
        m["caserow"] = ((1.0 - caseA) * -NEG)[None, :].astype(np.float32)
        m["brows"] = np.stack([b1, b2]).astype(np.float32)
        m["w2"] = np.ascontiguousarray(w2.astype(np.float32))
        in_maps.append(m)
    return in_maps


def kernel(q, k, mask_q, mask_k, Wq, Wk, Wv, Wu, bu):
    nc = _get_nc()
    in_maps = _host_prep(q, k, mask_q, mask_k, Wq, Wk, Wv, Wu, bu)
    res = run_bass_kernel_spmd(nc, in_maps, list(range(B)))
    out = np.stack([np.ascontiguousarray(res.results[b]["out"].T) for b in range(B)])
    return out.astype(np.float32)


# revision 8
# speedup vs baseline: 1.5293x; 1.0145x over previous
"""Multi-head causal+padded attention on 8 TRN2 NeuronCores.

Strategy: data-parallel over batch (8 batches -> 8 cores, no collectives).
Per core, everything is computed in a transposed layout so that no PE
transposes of the attention matrix are needed:

  QT[h] = (q Wq^T)^T slice  [e=128, tq]     KT[h] likewise
  V[kc] = (k Wv^T) row-chunk [tk=128, he]   (natural layout)
  S^T(kc,:) = KT[h][:,kc]^T-block matmuls   [tk-part, tq-free]
  A^T = exp(s * S^T + mk_bias)              (pad mask folded into exp bias,
                                             causal diag masked by a bf16
                                             identity x (-1e30 tri) matmul
                                             injected into PSUM)
  row sums  = ones^T @ A^T  (replicated across 128 partitions by all-ones lhsT)
  outT[h]   = sum_kc V[kc,h]^T-block @ A^T
  attnT[h]  = outT[h] * recip(sums)
  out^T     = sum_h WuT[h]^T @ attnT[h] + corr + bu

Degenerate softmax rows (all keys masked / no causal-visible key) are fixed
up exactly via two per-head mean-of-V vectors folded through the output
projection as a rank-2 correction (host-computed 0/1 row selectors).
Matmuls run as float32r (full PE rate at N>=256, near-fp32 precision).
"""

import numpy as np
import ml_dtypes

import concourse.bacc as bacc
import concourse.mybir as mybir
import concourse.tile as tile
from concourse.bass_utils import run_bass_kernel_spmd

F32 = mybir.dt.float32
F32R = mybir.dt.float32r
BF16 = mybir.dt.bfloat16
F16 = mybir.dt.float16

import os
SCORE_DT = {"f32r": F32R, "f16": F16}[os.environ.get("K_SCORE_DT", "f32r")]
SOFT_DT = {"f32r": F32R, "f16": F16}[os.environ.get("K_SOFT_DT", "f16")]
TRI_NEG = -60000.0 if SOFT_DT == F16 else -1.0e30
_SOFT_NP = {F16: "float16", F32R: "float32"}

B, TQ, TK, E, H = 8, 1024, 1024, 128, 8
HE = H * E
SCALE = float(E) ** -0.5
NEG = -1.0e30


def _chunks(kc):
    """Absolute column ranges for score row kc, split at the 512 PSUM bank."""
    lo = kc * 128
    if lo < 512:
        return [(lo, 512), (512, TQ)]
    return [(lo, TQ)]


def _build():
    nc = bacc.Bacc("TRN2", target_bir_lowering=False, debug=False)
    dp = nc.declare_dram_parameter
    d_qT = dp("qT", [E, TQ], F32R, isOutput=False)
    d_kT = dp("kT", [E, TK], F32R, isOutput=False)
    d_wqT = dp("wqT", [E, HE], F32R, isOutput=False)
    d_wkT = dp("wkT", [E, HE], F32R, isOutput=False)
    d_wvT = dp("wvT", [E, HE], F32R, isOutput=False)
    d_wuT = dp("wuT", [HE, E], SOFT_DT, isOutput=False)
    d_mkb = dp("mkbias", [128, 8], F32, isOutput=False)
    d_tri = dp("trineg", [128, 128], SOFT_DT, isOutput=False)
    d_idb = dp("identb", [128, 128], SOFT_DT, isOutput=False)
    d_case = dp("caserow", [1, TQ], F32R, isOutput=False)
    d_brow = dp("brows", [2, TQ], F32R, isOutput=False)
    d_w2 = dp("w2", [2, E], F32R, isOutput=False)
    d_ones1 = dp("onesk1", [1, 128], F32R, isOutput=False)
    d_ones = dp("ones128", [128, 128], SOFT_DT, isOutput=False)
    d_bu = dp("bu", [E, 1], F32, isOutput=False)
    d_out = dp("out", [E, TQ], F32, isOutput=True)

    Exp = mybir.ActivationFunctionType.Exp
    Ident = mybir.ActivationFunctionType.Identity
    mult = mybir.AluOpType.mult
    mm = nc.tensor.matmul

    with tile.TileContext(nc) as tc:
        with (
            tc.tile_pool(name="const", bufs=1) as cp,
            tc.tile_pool(name="persist", bufs=1) as pp,
        ):
            # ---- constants ----
            wu = []
            for h in range(H):
                t = cp.tile([128, 128], SOFT_DT, tag=f"wu{h}", name=f"wu{h}")
                nc.sync.dma_start(out=t[:], in_=d_wuT[h * 128 : (h + 1) * 128, :])
                wu.append(t)
            mkb = cp.tile([128, 8], F32, tag="mkb", name="mkb")
            nc.sync.dma_start(out=mkb[:], in_=d_mkb[:])
            tri = cp.tile([128, 128], SOFT_DT, tag="tri", name="tri")
            nc.sync.dma_start(out=tri[:], in_=d_tri[:])
            idb = cp.tile([128, 128], SOFT_DT, tag="idb", name="idb")
            nc.sync.dma_start(out=idb[:], in_=d_idb[:])
            case = cp.tile([1, TQ], F32R, tag="case", name="case")
            nc.sync.dma_start(out=case[:], in_=d_case[:])
            brow = cp.tile([2, TQ], F32R, tag="brow", name="brow")
            nc.sync.dma_start(out=brow[:], in_=d_brow[:])
            ones1 = cp.tile([1, 128], F32R, tag="ones1", name="ones1")
            nc.sync.dma_start(out=ones1[:], in_=d_ones1[:])
            ones = cp.tile([128, 128], SOFT_DT, tag="ones", name="ones")
            nc.sync.dma_start(out=ones[:], in_=d_ones[:])
            bu = cp.tile([E, 1], F32, tag="bu", name="bu")
            nc.sync.dma_start(out=bu[:], in_=d_bu[:])
            w2 = cp.tile([2, 128], F32R, tag="w2", name="w2")
            nc.sync.dma_start(out=w2[:], in_=d_w2[:])

            # ---- persistent activations ----
            QT = [pp.tile([128, TQ], SCORE_DT, tag=f"QT{h}", name=f"QT{h}") for h in range(H)]
            KT = [pp.tile([128, TK], SCORE_DT, tag=f"KT{h}", name=f"KT{h}") for h in range(H)]
            V = [pp.tile([128, HE], SOFT_DT, tag=f"V{kc}", name=f"V{kc}") for kc in range(8)]
            attnT = [
                pp.tile([128, TQ], SOFT_DT, tag=f"attnT{h}", name=f"attnT{h}")
                for h in range(H)
            ]

            # ---- phase 1: projections ----
            with (
                tc.tile_pool(name="proj", bufs=1) as jp,
                tc.tile_pool(name="ppsum", bufs=3, space="PSUM") as jps,
            ):
                qTs = jp.tile([E, TQ], F32R, tag="qTs", name="qTs")
                nc.sync.dma_start(out=qTs[:], in_=d_qT[:])
                kTs = jp.tile([E, TK], F32R, tag="kTs", name="kTs")
                nc.sync.dma_start(out=kTs[:], in_=d_kT[:])
                wq = jp.tile([E, HE], F32R, tag="wq", name="wq")
                nc.sync.dma_start(out=wq[:], in_=d_wqT[:])
                wk = jp.tile([E, HE], F32R, tag="wk", name="wk")
                nc.sync.dma_start(out=wk[:], in_=d_wkT[:])
                wv = jp.tile([E, HE], F32R, tag="wv", name="wv")
                nc.sync.dma_start(out=wv[:], in_=d_wvT[:])

                n_evac = 0

                def evac(dst, src):
                    nonlocal n_evac
                    if n_evac % 2 == 0:
                        nc.vector.tensor_copy(dst, src)
                    else:
                        nc.scalar.copy(dst, src)
                    n_evac += 1

                for h in range(H):
                    ps = jps.tile([128, TQ], F32, tag="pps", name=f"psq{h}")
                    for a, b in ((0, 512), (512, TQ)):
                        mm(ps[:, a:b], wq[:, h * 128 : (h + 1) * 128],
                           qTs[:, a:b], start=True, stop=True)
                    evac(QT[h][:], ps[:])
                for h in range(H):
                    ps = jps.tile([128, TK], F32, tag="pps", name=f"psk{h}")
                    for a, b in ((0, 512), (512, TK)):
                        mm(ps[:, a:b], wk[:, h * 128 : (h + 1) * 128],
                           kTs[:, a:b], start=True, stop=True)
                    evac(KT[h][:], ps[:])
                for kc in range(8):
                    ps = jps.tile([128, HE], F32, tag="pps", name=f"psv{kc}")
                    for a, b in ((0, 512), (512, HE)):
                        mm(ps[:, a:b], kTs[:, kc * 128 : (kc + 1) * 128],
                           wv[:, a:b], start=True, stop=True)
                    evac(V[kc][:], ps[:])


            # ---- phase 3: attention per head ----
            with (
                tc.tile_pool(name="stps", bufs=2, space="PSUM") as sp,
                tc.tile_pool(name="accps", bufs=1, space="PSUM") as ap_,
                tc.tile_pool(name="atp", bufs=3) as atp,
                tc.tile_pool(name="rbp", bufs=2) as rbp,
            ):
                state = {}

                def emit_epilogue(h):
                    # off-critical-path normalization for head h
                    sum_ps, out_ps = state[h]
                    last = h == H - 1
                    if last:
                        sum_src, out_src = sum_ps, out_ps
                    else:
                        sum_src = rbp.tile([128, TQ], F32, tag="sum_sb",
                                           name=f"ssb{h}")
                        nc.scalar.copy(sum_src[:], sum_ps[:])
                        out_src = rbp.tile([128, TQ], F32, tag="out_sb",
                                           name=f"osb{h}")
                        nc.vector.tensor_copy(out_src[:], out_ps[:])
                    rb = rbp.tile([128, TQ], F32, tag="rb", name=f"rb{h}")
                    nc.vector.reciprocal(out=rb[:], in_=sum_src[:])
                    nc.vector.tensor_tensor(
                        out=attnT[h][:], in0=out_src[:], in1=rb[:], op=mult
                    )

                for h in range(H):
                    sum_ps = ap_.tile([128, TQ], F32, tag="sum_ps", name=f"sum{h}")
                    out_ps = ap_.tile([128, TQ], F32, tag="out_ps", name=f"out{h}")
                    state[h] = (sum_ps, out_ps)
                    ats = [None] * 8

                    def consume(kc):
                        lo = kc * 128
                        for a, b in _chunks(kc):
                            ra, rb_ = a - lo, b - lo
                            mm(sum_ps[:, a:b], ones[:], ats[kc][:, ra:rb_],
                               start=(kc == 0), stop=False)
                            stop_pv = (kc == 3 and a < 512) or kc == 7
                            mm(out_ps[:, a:b],
                               V[kc][:, h * 128 : (h + 1) * 128],
                               ats[kc][:, ra:rb_], start=(kc == 0),
                               stop=stop_pv)

                    for kc in range(8):
                        lo = kc * 128
                        ncols = TQ - lo
                        ch = _chunks(kc)
                        st = sp.tile([128, TQ], F32, tag="st", name=f"st{h}_{kc}")
                        for i, (a, b) in enumerate(ch):
                            mm(st[:, a:b], KT[h][:, lo : lo + 128],
                               QT[h][:, a:b], start=True, stop=(i == 1))
                        mm(st[:, lo : lo + 128], idb[:], tri[:],
                           start=False, stop=True)
                        at = atp.tile([128, TQ], SOFT_DT, tag="at", name=f"at{h}_{kc}")
                        ats[kc] = at
                        nc.scalar.activation(
                            out=at[:, 0:ncols], in_=st[:, lo:TQ], func=Exp,
                            bias=mkb[:, kc : kc + 1], scale=SCALE,
                        )
                        if kc >= 1:
                            consume(kc - 1)
                        if kc == 1 and h >= 1:
                            emit_epilogue(h - 1)
                    consume(7)
                    for a, b in ((0, 512), (512, TQ)):
                        mm(sum_ps[:, a:b], ones1[:], case[:, a:b],
                           start=False, stop=True)
                emit_epilogue(H - 1)

            # ---- phase 4: output projection ----
            with tc.tile_pool(name="finps", bufs=1, space="PSUM") as fp:
                fin = fp.tile([128, TQ], F32, tag="fin", name="fin")
                for h in range(H):
                    for a, b in ((0, 512), (512, TQ)):
                        mm(fin[:, a:b], wu[h][:], attnT[h][:, a:b],
                           start=(h == 0), stop=False)
                for a, b in ((0, 512), (512, TQ)):
                    mm(fin[:, a:b], w2[:], brow[:, a:b],
                       start=False, stop=True)
                outsb = pp.tile([E, TQ], F32, tag="outsb", name="outsb")
                nc.scalar.activation(
                    out=outsb[:], in_=fin[:], func=Ident, bias=bu[:, 0:1], scale=1.0
                )
                nc.sync.dma_start(out=d_out[:], in_=outsb[:])

    nc.compile()
    return nc


_NC = None


def _get_nc():
    global _NC
    if _NC is None:
        _NC = _build()
    return _NC


def _host_prep(q, k, mask_q, mask_k, Wq, Wk, Wv, Wu, bu):
    shared = {
        "wqT": np.ascontiguousarray(Wq.T),
        "wkT": np.ascontiguousarray(Wk.T),
        "wvT": np.ascontiguousarray(Wv.T),
        "wuT": np.ascontiguousarray(Wu.T).astype(_SOFT_NP[SOFT_DT]),
        "trineg": (TRI_NEG * np.tril(np.ones((128, 128), np.float32), -1)).astype(_SOFT_NP[SOFT_DT]),
        "identb": np.eye(128).astype(_SOFT_NP[SOFT_DT]),
        "onesk1": np.ones((1, 128), np.float32),
        "ones128": np.ones((128, 128)).astype(_SOFT_NP[SOFT_DT]),
        "bu": np.ascontiguousarray(bu[:, None]),
    }
    WuWv = (Wu @ Wv).astype(np.float32)
    in_maps = []
    for b in range(B):
        mq = mask_q[b, :, 0].astype(np.float32)
        mk = mask_k[b, :, 0].astype(np.float32)
        c01 = (np.cumsum(mk) >= 1.0).astype(np.float32)
        caseA = mq * c01
        b1 = mq * (1.0 - c01)
        b2 = 1.0 - mq
        s1m = 1.0 - mk
        denom = max(float(s1m.sum()), 1.0)
        wvecs = np.stack([s1m / denom, np.full(TK, 1.0 / TK, np.float32)], axis=1)
        w2 = (wvecs.T.astype(np.float32) @ k[b]) @ WuWv.T
        m = dict(shared)
        m["qT"] = np.ascontiguousarray(q[b].T)
        m["kT"] = np.ascontiguousarray(k[b].T)
        m["mkbias"] = np.ascontiguousarray(
            ((mk - 1.0) * -NEG).reshape(8, 128).T
        ).astype(np.float32)
        m["caserow"] = ((1.0 - caseA) * -NEG)[None, :].astype(np.float32)
        m["brows"] = np.stack([b1, b2]).astype(np.float32)
        m["w2"] = np.ascontiguousarray(w2.astype(np.float32))
        in_maps.append(m)
    return in_maps


def kernel(q, k, mask_q, mask_k, Wq, Wk, Wv, Wu, bu):
    nc = _get_nc()
    in_maps = _host_prep(q, k, mask_q, mask_k, Wq, Wk, Wv, Wu, bu)
    res = run_bass_kernel_spmd(nc, in_maps, list(range(B)))
    out = np.stack([np.ascontiguousarray(res.results[b]["out"].T) for b in range(B)])
    return out.astype(np.float32)
